# revision 5
# baseline (speedup 1.0000x reference)
"""Trainium2 Bass kernel for ClusterRetrieval (retrieval_knn).

reference semantics (B=16384, N=16384, D=128, k=8):
    sims = Q @ C.T                      # [B, N] f32
    vals, idx = top_k(sims, 8)          # descending, ties -> lowest index
    ids = where(vals >= 0.0, idx, -1)   # int32
    return (ids, vals, sims)

Sharding: data-parallel over the 8 NeuronCores — Q split into 8 row shards
of 2048, centroids replicated. Each core computes its [2048, 16384] simi-
larity block (PE matmul, D=128 contraction on partitions), streams it to
HBM, and runs DVE max8/max_index for the per-row top-8.
"""

import os
import sys

import numpy as np

for _p in ("/root/.axon_site", "/root/.axon_site/_ro/trn_rl_repo", "/opt/trn_rl_repo"):
    if os.path.isdir(_p) and _p not in sys.path:
        sys.path.append(_p)

B, N, D, K = 16384, 16384, 128, 8
N_CORES = 8
B_SH = B // N_CORES          # 2048 rows per core
P = 128                      # partitions
NT_Q = B_SH // P             # 16 query tiles per core
MM_N = 512                   # moving free dim per matmul
PS_W = 2048                  # psum tile width (4 banks)
N_PS = N // PS_W             # 8 psum passes per query tile

_CACHE = {}
last_run_info = {}


def _build_program():
    import concourse.mybir as mybir
    from concourse import bacc
    from concourse.masks import make_identity
    from concourse.tile import TileContext

    f32 = mybir.dt.float32
    i32 = mybir.dt.int32
    u32 = mybir.dt.uint32

    nc = bacc.Bacc("TRN2", target_bir_lowering=False)
    q = nc.dram_tensor("q", [B_SH, D], f32, kind="ExternalInput")
    c = nc.dram_tensor("c", [N, D], f32, kind="ExternalInput")
    sims = nc.dram_tensor("sims", [B_SH, N], f32, kind="ExternalOutput")
    vals = nc.dram_tensor("vals", [B_SH, K], f32, kind="ExternalOutput")
    ids = nc.dram_tensor("ids", [B_SH, K], i32, kind="ExternalOutput")

    with TileContext(nc) as tc:
        with (
            tc.tile_pool(name="const", bufs=1) as const_pool,
            tc.tile_pool(name="ct", bufs=1) as ct_pool,
            tc.tile_pool(name="qt", bufs=1) as qt_pool,
        ):
            ident = const_pool.tile([P, P], f32)
            make_identity(nc, ident)
            neg1 = const_pool.tile([P, K], i32)
            nc.vector.memset(neg1, -1)

            ct = ct_pool.tile([P, N], f32)     # C^T  (d on partitions)
            qt = qt_pool.tile([P, B_SH], f32)  # Q^T  (d on partitions)

            # ---- prologue: load + transpose C and Q into SBUF ----
            with (
                tc.tile_pool(name="ld", bufs=6) as ld_pool,
                tc.tile_pool(name="tp", bufs=4, space="PSUM") as tp_pool,
            ):
                # load 512 rows per DMA (3D AP: 4 x [128, 128] tiles)
                for i in range(N // 512):
                    lt = ld_pool.tile([P, 512], f32, tag="ld")
                    src = c[i * 512:(i + 1) * 512, :].rearrange(
                        "(g p) d -> p g d", p=P
                    )
                    nc.sync.dma_start(lt.rearrange("p (g d) -> p g d", g=4), src)
                    for g in range(4):
                        pt = tp_pool.tile([P, P], f32)
                        nc.tensor.transpose(pt, lt[:, g * P:(g + 1) * P], ident)
                        nc.any.tensor_copy(
                            ct[:, (i * 4 + g) * P:(i * 4 + g + 1) * P], pt
                        )
                for i in range(B_SH // 512):
                    lt = ld_pool.tile([P, 512], f32, tag="ld")
                    src = q[i * 512:(i + 1) * 512, :].rearrange(
                        "(g p) d -> p g d", p=P
                    )
                    nc.sync.dma_start(lt.rearrange("p (g d) -> p g d", g=4), src)
                    for g in range(4):
                        pt = tp_pool.tile([P, P], f32)
                        nc.tensor.transpose(pt, lt[:, g * P:(g + 1) * P], ident)
                        nc.any.tensor_copy(
                            qt[:, (i * 4 + g) * P:(i * 4 + g + 1) * P], pt
                        )

            # ---- main loop: matmul -> copy -> (dma out | top8) ----
            with (
                tc.tile_pool(name="sim", bufs=2) as sim_pool,
                tc.tile_pool(name="ps", bufs=2, space="PSUM") as ps_pool,
                tc.tile_pool(name="tk", bufs=3) as tk_pool,
            ):
                for ti in range(NT_Q):
                    r0 = ti * P
                    st = sim_pool.tile([P, N], f32, tag="st")
                    lhsT = qt[:, r0:r0 + P]
                    for pi in range(N_PS):
                        pt = ps_pool.tile([P, PS_W], f32, tag="ps")
                        for bi in range(PS_W // MM_N):
                            n0 = bi * MM_N
                            nc.tensor.matmul(
                                pt[:, n0:n0 + MM_N],
                                lhsT=lhsT,
                                rhs=ct[:, pi * PS_W + n0:pi * PS_W + n0 + MM_N],
                                start=True,
                                stop=True,
                            )
                        nc.any.tensor_copy(
                            st[:, pi * PS_W:(pi + 1) * PS_W], pt
                        )
                        nc.sync.dma_start(
                            sims[r0:r0 + P, pi * PS_W:(pi + 1) * PS_W],
                            st[:, pi * PS_W:(pi + 1) * PS_W],
                        )
                    v = tk_pool.tile([P, K], f32, tag="v")
                    nc.vector.max(v, st)
                    ix = tk_pool.tile([P, K], u32, tag="ix")
                    nc.vector.max_index(ix, v, st)
                    di = tk_pool.tile([P, K], i32, tag="di")
                    nc.vector.tensor_copy(di, ix)
                    m = tk_pool.tile([P, K], u32, tag="m")
                    nc.vector.tensor_scalar(
                        m, v, 0.0, None, op0=mybir.AluOpType.is_lt
                    )
                    nc.vector.copy_predicated(di, m, neg1)
                    nc.sync.dma_start(vals[r0:r0 + P, :], v)
                    nc.sync.dma_start(ids[r0:r0 + P, :], di)

    nc.compile()
    return nc


def _get_program():
    if "nc" not in _CACHE:
        _CACHE["nc"] = _build_program()
    return _CACHE["nc"]


def run_sharded(q_full, c_full, trace=False, **spmd_kwargs):
    from concourse.bass_utils import run_bass_kernel_spmd

    nc = _get_program()
    q_full = np.ascontiguousarray(q_full, dtype=np.float32)
    c_full = np.ascontiguousarray(c_full, dtype=np.float32)
    in_maps = [
        {"q": q_full[i * B_SH:(i + 1) * B_SH], "c": c_full}
        for i in range(N_CORES)
    ]
    res = run_bass_kernel_spmd(
        nc, in_maps, core_ids=list(range(N_CORES)), trace=trace, **spmd_kwargs
    )
    last_run_info["exec_time_ns"] = res.exec_time_ns
    last_run_info["mean_exec_time_ns"] = res.mean_exec_time_ns
    last_run_info["trace"] = res.instructions_and_trace
    sims = np.concatenate([r["sims"] for r in res.results], axis=0)
    vals = np.concatenate([r["vals"] for r in res.results], axis=0)
    ids = np.concatenate([r["ids"] for r in res.results], axis=0)
    return ids, vals, sims


def kernel(query_embeddings, cluster_embeddings, top_k):
    k = min(int(top_k), N)
    assert k <= K, f"kernel compiled for top-{K}, got top_k={k}"
    ids, vals, sims = run_sharded(query_embeddings, cluster_embeddings)
    if k < K:
        ids, vals = ids[:, :k], vals[:, :k]
    return ids, vals, sims
